# revision 65
# baseline (speedup 1.0000x reference)
"""Trainium2 Bass kernel for nn_DiscreteDecisionTransformer (v2).

Decision-transformer forward: embed(a,r,s) -> LN -> +posenc, then 4 blocks of
[causal self-attn, cross-attn, FFN] with post-LN, then action head.

Distribution: data-parallel over batch, 16 batches / 8 cores = 2 per core.
Params replicated; zero collectives. Feature-major layout ([dmodel on
partitions, tokens on free]) so GEMMs contract over partitions.

v2 changes vs the original working kernel (sim: 3.09ms -> 2.50ms):
 - Single-role residual stream: LayerNorm affine + residual adds are in-place,
   halving the X footprint.
 - LN betas folded into downstream linear biases wherever the next consumer is
   linear (LN0->block0 qkv + bo, LN2->b1/b2, LN3->next qkv + bo, last->fc_b).
   Only LN1 keeps an explicit beta (it feeds LN2, a nonlinearity).
 - V-projection bias folded into bo (bv @ Wo) -- the softmax denominator trick
   makes per-token V-bias contribution = bv exactly; r-embedding done as a
   rank-1 broadcast*scale on ACT instead of a matmul.
 - Batched DMAs: per-block weights land in 1 DMA each ([128, kt, cols] APs);
   FFN weights stream in 16 DMAs per (batch, chunk) instead of 120. This cut
   the serial HWDGE descriptor-generation time from 1.36ms to 0.23ms.
 - PSUM->SBUF bias-adds moved to the Activation engine (per-partition bias AP),
   LN affine and stats run bf16; 1/D folded into the stats matmul constant.
 - Emission-order tuning so in-order engine queues overlap: Q projected
   per-chunk inside attention, LN3 deferred into the next block's prologue,
   (batch, chunk) pairs interleaved through LN1/LN2/FFN, split A/B gpsimd
   broadcasts. Embed-phase scratch aliases the idle FFN hidden-tile pool.

GEMMs in bf16 with f32 PSUM accumulation (fp8 DoubleRow was measured at
rel-err 3.9e-2 vs the 2e-2 budget -- not viable). End-to-end error vs the
f32 reference is ~7e-3.
"""

import sys
from contextlib import ExitStack

sys.path.insert(0, "/opt/trn_rl_repo")

import numpy as np
import ml_dtypes

import concourse.bacc as bacc
import concourse.mybir as mybir
import concourse.tile as tile
from concourse.bass_utils import run_bass_kernel_spmd

bf = ml_dtypes.bfloat16

B, L, D, H, DH, NB, E = 16, 1024, 768, 8, 96, 4, 256
A_DIM, S_DIM = 64, 128
NCORES = 8
CPC = B // NCORES  # batches per core
KT = D // 128      # 6 k-tiles of dmodel
MT = D // 128      # 6 m-tiles of dmodel
CH = 512           # token chunk for attention / LN
NCH = L // CH      # 2 chunks per batch
FFT = 4 * D // 128 # 24 m-tiles of ffn hidden
F32, BF = mybir.dt.float32, mybir.dt.bfloat16
AL = mybir.AluOpType
AF = mybir.ActivationFunctionType

_CACHE = {}

# bmeta column layout (f32, per block): per-partition packed scalars
_BQ = 0          # [96, 8]   q bias (scaled)
_BK = 8          # [96, 8]   k bias
_BO = 16         # [128, 6]  o bias (+bv@Wo +beta_in)
_B2 = 22         # [128, 6]  ffn2 bias (+beta2)
_CAB = 28        # [128, 6*CPC] cross-attn bias + ln1 beta, per batch
_L1G = 40        # [128, 6]  ln1 gamma
_L1N = 46        # [128, 6]  -ln1 gamma
_L2G = 52
_L2N = 58
_L3G = 64
_L3N = 70
_B1 = 76         # [128, 24] ffn1 bias (+beta2@W1)
_BMETA_COLS = 100


def _rearr_pk(ap, p):
    return ap.rearrange("(k p) -> p k", p=p)


def _build(reps=1):
    nc = bacc.Bacc("TRN2", target_bir_lowering=False, debug=False)
    dram = nc.dram_tensor

    ars = dram("ars", [CPC, 193, L], BF, kind="ExternalInput")
    wa = dram("wa", [A_DIM, E], BF, kind="ExternalInput")
    wrp = dram("wrp", [128, 2], F32, kind="ExternalInput")
    ws = dram("ws", [S_DIM, E], BF, kind="ExternalInput")
    bemb = dram("bemb", [D], F32, kind="ExternalInput")
    lnp0 = dram("lnp0", [2, D], F32, kind="ExternalInput")  # g, -g (beta folded)
    pos = dram("pos", [128, KT, L], BF, kind="ExternalInput")
    wq = dram("wq", [NB, 128, KT, D], BF, kind="ExternalInput")
    wk = dram("wk", [NB, 128, KT, D], BF, kind="ExternalInput")
    wv = dram("wv", [NB, 128, KT, D], BF, kind="ExternalInput")
    wo = dram("wo", [NB, DH, H, D], BF, kind="ExternalInput")
    w1 = dram("w1", [NB, 128, KT, 4 * D], BF, kind="ExternalInput")
    w2 = dram("w2", [NB, 128, FFT, D], BF, kind="ExternalInput")
    bmeta = dram("bmeta", [NB, 128, _BMETA_COLS], F32, kind="ExternalInput")
    masks = dram("masks", [128, 896], BF, kind="ExternalInput")
    fcw = dram("fcw", [D, A_DIM], BF, kind="ExternalInput")
    fcb = dram("fcb", [A_DIM], F32, kind="ExternalInput")
    y = dram("y", [CPC, A_DIM, L], F32, kind="ExternalOutput")

    with nc.allow_low_precision(reason="bf16 kernel by design"), \
         tile.TileContext(nc) as tc, ExitStack() as ctx:
            ep = ctx.enter_context
            cst = ep(tc.tile_pool(name="cst", bufs=1))
            wblk = ep(tc.tile_pool(name="wblk", bufs=1))
            bmp = ep(tc.tile_pool(name="bmp", bufs=2))
            w1p = ep(tc.tile_pool(name="w1p", bufs=4))
            w2p = ep(tc.tile_pool(name="w2p", bufs=3))
            xp = ep(tc.tile_pool(name="xp", bufs=1))
            qkp = ep(tc.tile_pool(name="qk", bufs=1))
            vap = ep(tc.tile_pool(name="vap", bufs=1))
            ptp = ep(tc.tile_pool(name="ptp", bufs=8))
            otp = ep(tc.tile_pool(name="otp", bufs=1))
            scr = ep(tc.tile_pool(name="scr", bufs=3))
            hp = ep(tc.tile_pool(name="hp", bufs=1))
            smv = ep(tc.tile_pool(name="smv", bufs=3))
            abp = ep(tc.tile_pool(name="abp", bufs=2))
            bias = ep(tc.tile_pool(name="bias", bufs=1))
            pmm = ep(tc.tile_pool(name="pmm", bufs=6, space="PSUM"))

            # ---------- global constants ----------
            ones = cst.tile([128, 1], BF)
            nc.any.memset(ones[:], 1.0)
            ones_d = cst.tile([128, 1], BF)
            nc.any.memset(ones_d[:], 1.0 / D)
            epst = cst.tile([1, 1], F32)
            nc.any.memset(epst[:], 1e-5)
            bigm = cst.tile([128, 896], BF, tag="bigm")
            nc.sync.dma_start(bigm[:], masks[:])
            maskt = [bigm[:, 384 - rt * 128:896 - rt * 128] for rt in range(4)]
            fct = cst.tile([128, KT, A_DIM], BF, tag="fcw")
            nc.sync.dma_start(fct[:], fcw[:].rearrange("(k p) m -> p k m", p=128))
            fcbt = cst.tile([A_DIM, 1], F32, tag="fcb")
            nc.sync.dma_start(fcbt[:], fcb[:].rearrange("(m o) -> m o", o=1))

            # residual-stream tiles (single role, in-place updates)
            xt = [[xp.tile([128, L], BF, tag=f"x{b}_{k}", name=f"x{b}_{k}")
                   for k in range(KT)] for b in range(CPC)]

            def ln_chunk(c, X, gt, gnt, bt_=None, post_pos=False):
                """In-place LayerNorm (no beta unless bt_ given) over features
                for one 512-token chunk. X: 6 [128, L] bf16 tiles."""
                cs = slice(c * CH, (c + 1) * CH)
                if post_pos:
                    # embed-only: borrow the (idle) FFN hidden tiles
                    postc = [hp.tile([128, CH], BF, tag=f"h{k}",
                                     name=f"pos{k}") for k in range(KT)]
                    for k in range(KT):
                        nc.sync.dma_start(postc[k][:], pos[:, k, cs])
                # shares the PSUM bank with attention's `po` (phase-disjoint)
                st = pmm.tile([DH + 1, CH], F32, tag="po", bufs=2)
                # ones_d = 1/D, so the matmuls produce mu / E[x^2] directly
                for k in range(KT):
                    nc.tensor.matmul(st[0:1, :], ones_d[:], X[k][:, cs],
                                     start=(k == 0), stop=(k == KT - 1))
                for k in range(KT):
                    xsq = scr.tile([128, CH], BF, tag="xsq", bufs=2)
                    nc.scalar.activation(xsq[:], X[k][:, cs], AF.Square)
                    nc.tensor.matmul(st[32:33, :], ones_d[:], xsq[:],
                                     start=(k == 0), stop=(k == KT - 1))
                musm = smv.tile([1, 2 * CH], F32, tag="musm", bufs=2)
                nc.vector.tensor_copy(musm[:, 0:CH], st[0:1, :])
                nc.vector.tensor_copy(musm[:, CH:2 * CH], st[32:33, :])
                mu = musm[:, 0:CH]
                mu2 = smv.tile([1, CH], F32, tag="sm")
                nc.vector.tensor_mul(mu2[:], mu, mu)
                var = smv.tile([1, CH], F32, tag="sm")
                nc.vector.tensor_sub(var[:], musm[:, CH:2 * CH], mu2[:])
                sd = smv.tile([1, CH], F32, tag="sm")
                nc.scalar.activation(sd[:], var[:], AF.Sqrt, bias=epst[:])
                ab = abp.tile([1, 2 * CH], BF, tag="ab", bufs=1)
                nc.vector.reciprocal(ab[:, 0:CH], sd[:])
                abb = abp.tile([128, 2 * CH], BF, tag="abb")
                nc.gpsimd.partition_broadcast(abb[:, 0:CH], ab[:, 0:CH])
                nc.vector.tensor_mul(ab[:, CH:2 * CH], mu, ab[:, 0:CH])
                nc.gpsimd.partition_broadcast(abb[:, CH:2 * CH],
                                              ab[:, CH:2 * CH])
                for k in range(KT):
                    u = scr.tile([128, CH], BF, tag="scr")
                    nc.vector.scalar_tensor_tensor(
                        u[:], X[k][:, cs], gt[:, k:k + 1], abb[:, 0:CH],
                        op0=AL.mult, op1=AL.mult)
                    if post_pos:
                        w_ = scr.tile([128, CH], BF, tag="scr")
                        nc.vector.scalar_tensor_tensor(
                            w_[:], abb[:, CH:2 * CH], gnt[:, k:k + 1], u[:],
                            op0=AL.mult, op1=AL.add)
                        nc.vector.tensor_add(X[k][:, cs], w_[:],
                                             postc[k][:])
                    else:
                        nc.vector.scalar_tensor_tensor(
                            X[k][:, cs], abb[:, CH:2 * CH], gnt[:, k:k + 1],
                            u[:], op0=AL.mult, op1=AL.add)
                        if bt_ is not None:
                            nc.scalar.activation(X[k][:, cs], X[k][:, cs],
                                                 AF.Identity,
                                                 bias=bt_[:, k:k + 1])

            def emit_forward():
                # ---------- embed + LN + posenc ----------
                wat = cst.tile([A_DIM, E], BF, tag="wa")
                nc.sync.dma_start(wat[:], wa[:])
                wrt = cst.tile([128, 2], F32, tag="wr")
                nc.sync.dma_start(wrt[:], wrp[:])
                wst = cst.tile([S_DIM, E], BF, tag="ws")
                nc.sync.dma_start(wst[:], ws[:])
                bembt = cst.tile([128, KT], F32, tag="bemb")
                nc.sync.dma_start(bembt[:], _rearr_pk(bemb[:], 128))
                p0 = []
                for j in range(2):
                    t = cst.tile([128, KT], F32, tag=f"lnp0{j}")
                    nc.sync.dma_start(t[:], _rearr_pk(lnp0[j], 128))
                    p0.append(t)

                for b in range(CPC):
                    X = xt[b]
                    # embed inputs borrow idle FFN hidden tiles / ab tile
                    tr = abp.tile([1, 2 * CH], BF, tag="ab", bufs=1,
                                  name="tr")
                    nc.sync.dma_start(tr[:], ars[b, A_DIM:A_DIM + 1, :])
                    for c in range(NCH):
                        cc = slice(c * CH, (c + 1) * CH)
                        ta = hp.tile([128, CH], BF, tag="h6", name="ta")
                        nc.sync.dma_start(ta[0:A_DIM, :], ars[b, 0:A_DIM, cc])
                        ts = hp.tile([128, CH], BF, tag="h7", name="ts")
                        nc.sync.dma_start(ts[:], ars[b, A_DIM + 1:193, cc])
                        rb = hp.tile([128, CH], BF, tag="h8", name="rb")
                        nc.gpsimd.partition_broadcast(rb[:], tr[:, cc])
                        for m in range(MT):
                            ms = slice((m % 2) * 128, (m % 2) * 128 + 128)
                            if 2 <= m < 4:
                                # r-embedding is rank-1: X[m] = r*wr + bemb
                                nc.scalar.activation(
                                    X[m][:, cc], rb[:], AF.Identity,
                                    bias=bembt[:, m:m + 1],
                                    scale=wrt[:, m - 2:m - 1])
                                continue
                            p = pmm.tile([128, CH], F32, tag="mm")
                            if m < 2:
                                nc.tensor.matmul(p[:], wat[:, ms],
                                                 ta[0:A_DIM, :],
                                                 start=True, stop=True)
                            else:
                                nc.tensor.matmul(p[:], wst[:, ms], ts[:],
                                                 start=True, stop=True)
                            nc.scalar.activation(X[m][:, cc], p[:],
                                                 AF.Identity,
                                                 bias=bembt[:, m:m + 1])
                    for c in range(NCH):
                        ln_chunk(c, X, p0[0], p0[1], post_pos=True)

                # ---------- transformer blocks ----------
                # LN3 of batch b is emitted lazily, right before the next
                # consumer of X[b], so its DVE chain overlaps other batches'
                # PE work instead of stalling the in-order PE queue.
                ln3_pending = [None] * CPC
                for i in range(NB):
                    wqt = wblk.tile([128, KT, D], BF, tag="wq")
                    nc.sync.dma_start(wqt[:], wq[i])
                    wkt = wblk.tile([128, KT, D], BF, tag="wk")
                    nc.sync.dma_start(wkt[:], wk[i])
                    wvt = wblk.tile([128, KT, D], BF, tag="wv")
                    nc.sync.dma_start(wvt[:], wv[i])
                    wot = wblk.tile([DH, H, D], BF, tag="wo")
                    nc.sync.dma_start(wot[:], wo[i])
                    bm = bmp.tile([128, _BMETA_COLS], F32, tag="bm")
                    nc.sync.dma_start(bm[:], bmeta[i])

                    # flush deferred LN3s up front: b0's chain must finish
                    # before K-proj(b0), but b1's overlaps attention(b0)
                    for b in range(CPC):
                        if ln3_pending[b] is not None:
                            pg, pn = ln3_pending[b]
                            for c in range(NCH):
                                ln_chunk(c, xt[b], pg, pn)
                            ln3_pending[b] = None

                    for b in range(CPC):
                        X = xt[b]
                        # ---- K projection, full L (Q is done per chunk) ----
                        kt_ = []
                        for h in range(H):
                            tk = qkp.tile([DH, L], BF, tag=f"k{h}")
                            kt_.append(tk)
                            hs = slice(h * DH, (h + 1) * DH)
                            for c in range(NCH):
                                cc = slice(c * CH, (c + 1) * CH)
                                pk = pmm.tile([DH, CH], F32, tag="mm")
                                for k in range(KT):
                                    nc.tensor.matmul(pk[:], wkt[:, k, hs],
                                                     X[k][:, cc],
                                                     start=(k == 0),
                                                     stop=(k == KT - 1))
                                nc.scalar.activation(
                                    tk[:, cc], pk[:], AF.Identity,
                                    bias=bm[0:DH, _BK + h:_BK + h + 1])
                        # ---- V projection (token-major, bias folded away) --
                        vt = []
                        for tg in range(L // 128):
                            tok = slice(tg * 128, (tg + 1) * 128)
                            tv = vap.tile([128, 8 * 97], BF, tag=f"v{tg}")
                            for hg in range(2):
                                pv = pmm.tile([128, 4 * DH], F32, tag="mm")
                                for k in range(KT):
                                    nc.tensor.matmul(
                                        pv[:], X[k][:, tok],
                                        wvt[:, k, hg * 4 * DH:(hg + 1) * 4 * DH],
                                        start=(k == 0), stop=(k == KT - 1))
                                nc.scalar.activation(
                                    tv[:, hg * 4 * 97:(hg + 1) * 4 * 97]
                                    .rearrange("p (h d) -> p h d", d=97)
                                    [:, :, 0:DH],
                                    pv[:].rearrange("p (h d) -> p h d", d=DH),
                                    AF.Identity)
                            nc.vector.memset(tv[:, 96:8 * 97:97], 1.0)
                            vt.append(tv)
                        # ---- attention + O-proj per chunk ----
                        for c in range(NCH):
                            cs = slice(c * CH, (c + 1) * CH)
                            ktc = 4 * (c + 1)
                            ot = []
                            for h in range(H):
                                hs = slice(h * DH, (h + 1) * DH)
                                pq = pmm.tile([DH, CH], F32, tag="mm")
                                for k in range(KT):
                                    nc.tensor.matmul(pq[:], wqt[:, k, hs],
                                                     X[k][:, cs],
                                                     start=(k == 0),
                                                     stop=(k == KT - 1))
                                tq = qkp.tile([DH, CH], BF, tag="q", bufs=4)
                                nc.scalar.activation(
                                    tq[:], pq[:], AF.Identity,
                                    bias=bm[0:DH, _BQ + h:_BQ + h + 1])
                                po = pmm.tile([DH + 1, CH], F32, tag="po",
                                              bufs=2)
                                for g0 in range(0, ktc, 4):
                                    pts = []
                                    for kt2 in range(g0, g0 + 4):
                                        ks2 = slice(kt2 * 128, (kt2 + 1) * 128)
                                        ptile = ptp.tile([128, CH], BF,
                                                         tag="pt")
                                        rt = kt2 - 4 * c
                                        if rt < 0:
                                            psc = pmm.tile([128, CH], F32,
                                                           tag="mm")
                                            nc.tensor.matmul(
                                                psc[:], kt_[h][:, ks2], tq[:],
                                                start=True, stop=True)
                                            nc.scalar.activation(
                                                ptile[:], psc[:], AF.Exp)
                                            pts.append(ptile)
                                            continue
                                        psc = pmm.tile([128, CH], F32,
                                                       tag="mm")
                                        nc.tensor.matmul(
                                            psc[:], kt_[h][:, ks2], tq[:],
                                            start=True, stop=True)
                                        tmp = scr.tile([128, CH], F32,
                                                       tag="scrf", bufs=2)
                                        nc.vector.scalar_tensor_tensor(
                                            tmp[:], psc[:], 1.0, maskt[rt],
                                            op0=AL.mult, op1=AL.add)
                                        nc.scalar.activation(
                                            ptile[:], tmp[:], AF.Exp)
                                        pts.append(ptile)
                                    for j, ptile in enumerate(pts):
                                        kt2 = g0 + j
                                        nc.tensor.matmul(
                                            po[:],
                                            vt[kt2][:, h * 97:h * 97 + 97],
                                            ptile[:],
                                            start=(kt2 == 0),
                                            stop=(kt2 == ktc - 1))
                                dinv = abp.tile([1, CH], BF, tag="dv",
                                                name="dinv", bufs=1)
                                nc.vector.reciprocal(dinv[:], po[DH:DH + 1, :])
                                dib = abp.tile([DH, CH], BF, tag="dib")
                                nc.gpsimd.partition_broadcast(dib[:], dinv[:])
                                oht = otp.tile([DH, CH], BF, tag=f"o{h}",
                                               name=f"o{h}")
                                nc.vector.scalar_tensor_tensor(
                                    oht[:], po[0:DH, :], 1.0, dib[:],
                                    op0=AL.mult, op1=AL.mult)
                                ot.append(oht)
                            for m in range(MT):
                                ms = slice(m * 128, (m + 1) * 128)
                                pp = pmm.tile([128, CH], F32, tag="mm")
                                for h in range(H):
                                    nc.tensor.matmul(pp[:], wot[:, h, ms],
                                                     ot[h][:],
                                                     start=(h == 0),
                                                     stop=(h == H - 1))
                                nc.vector.scalar_tensor_tensor(
                                    X[m][:, cs], pp[:], bm[:, _BO + m:_BO + m + 1],
                                    X[m][:, cs], op0=AL.add, op1=AL.add)
                    # interleave (b, c) pairs: each FFN's matmuls overlap the
                    # next pair's LN chains on the vector engine. b0's LN3 is
                    # emitted mid-block (overlapping FFN(b1,c1) matmuls); b1's
                    # stays deferred to the next block's prologue.
                    for b, c in ((0, 0), (1, 0), (0, 1), (1, 1)):
                        X = xt[b]
                        if True:
                            cs = slice(c * CH, (c + 1) * CH)
                            # LN1 (beta = cross-attn bias + ln1_b), in place
                            ln_chunk(c, X, bm[:, _L1G:_L1G + KT],
                                     bm[:, _L1N:_L1N + KT],
                                     bt_=bm[:, _CAB + b * KT:_CAB + (b + 1) * KT])
                            # LN2 (beta folded into b1/b2), in place
                            ln_chunk(c, X, bm[:, _L2G:_L2G + KT],
                                     bm[:, _L2N:_L2N + KT])
                            # ---- FFN ----
                            ht = [hp.tile([128, CH], BF, tag=f"h{m}",
                                          name=f"h{m}") for m in range(FFT)]
                            for mg in range(FFT // 2):
                                colg = slice(mg * 256, (mg + 1) * 256)
                                w1g = w1p.tile([128, KT, 256], BF, tag="w1")
                                nc.sync.dma_start(w1g[:], w1[i, :, :, colg])
                                for mi in range(2):
                                    m = mg * 2 + mi
                                    p1 = pmm.tile([128, CH], F32, tag="mm")
                                    for k in range(KT):
                                        nc.tensor.matmul(
                                            p1[:],
                                            w1g[:, k, mi * 128:(mi + 1) * 128],
                                            X[k][:, cs],
                                            start=(k == 0), stop=(k == KT - 1))
                                    nc.scalar.activation(
                                        ht[m][:], p1[:], AF.Relu,
                                        bias=bm[:, _B1 + m:_B1 + m + 1])
                            for grp in range(2):
                                mcols = slice(grp * 384, (grp + 1) * 384)
                                p2s = [pmm.tile([128, CH], F32, tag="mm",
                                                name=f"p2_{mi}")
                                       for mi in range(3)]
                                for j in range(4):
                                    w2g = w2p.tile([128, FFT // 4, 384], BF,
                                                   tag="w2")
                                    nc.sync.dma_start(
                                        w2g[:],
                                        w2[i][:, j * 6:(j + 1) * 6, mcols])
                                    for k6 in range(FFT // 4):
                                        k = j * 6 + k6
                                        for mi in range(3):
                                            nc.tensor.matmul(
                                                p2s[mi][:],
                                                w2g[:, k6,
                                                    mi * 128:(mi + 1) * 128],
                                                ht[k][:],
                                                start=(k == 0),
                                                stop=(k == FFT - 1))
                                for mi in range(3):
                                    m = grp * 3 + mi
                                    nc.vector.scalar_tensor_tensor(
                                        X[m][:, cs], p2s[mi][:],
                                        bm[:, _B2 + m:_B2 + m + 1],
                                        X[m][:, cs], op0=AL.add, op1=AL.add)
                        # LN3 deferred (beta folded into next block / fc)
                        ln3_pending[b] = (bm[:, _L3G:_L3G + KT],
                                          bm[:, _L3N:_L3N + KT])

                # ---------- action head ----------
                for b in range(CPC):
                    X = xt[b]
                    if ln3_pending[b] is not None:
                        pg, pn = ln3_pending[b]
                        for c in range(NCH):
                            ln_chunk(c, X, pg, pn)
                        ln3_pending[b] = None
                    for c in range(NCH):
                        cs = slice(c * CH, (c + 1) * CH)
                        pf = pmm.tile([A_DIM, CH], F32, tag="mm")
                        for k in range(KT):
                            nc.tensor.matmul(pf[:], fct[:, k, :], X[k][:, cs],
                                             start=(k == 0), stop=(k == KT - 1))
                        yt = scr.tile([128, CH], F32, tag="scrf", bufs=2)
                        nc.vector.tensor_scalar_add(yt[0:A_DIM, :], pf[:],
                                                    fcbt[:])
                        nc.sync.dma_start(y[b, :, cs], yt[0:A_DIM, :])

            for _rep in range(reps):
                emit_forward()

    nc.compile()
    return nc


def _posenc(length, d):
    pos_ = np.arange(length, dtype=np.float32)[:, None]
    i = np.arange(0, d, 2, dtype=np.float32)[None, :]
    ang = pos_ / np.power(np.float32(10000.0), i / np.float32(d))
    pe = np.zeros((length, d), np.float32)
    pe[:, 0::2] = np.sin(ang)
    pe[:, 1::2] = np.cos(ang)
    return pe


def _ksplit(w, p=128):
    """[K, F] -> [p, K//p, F] with k = t*p + row."""
    k, f = w.shape
    return np.ascontiguousarray(w.reshape(k // p, p, f).transpose(1, 0, 2))


def _host_prep(inp):
    f32 = np.float32
    a, r, s, t = (np.asarray(inp[k]) for k in ("a", "r", "s", "t"))
    ars = np.concatenate(
        [np.asarray(a, f32), np.asarray(r, f32), np.asarray(s, f32)],
        axis=-1).transpose(0, 2, 1)  # [B, 193, L]
    ars = np.ascontiguousarray(ars).astype(bf)

    scale = f32(1.0 / np.sqrt(DH))
    sa_Wqkv = np.asarray(inp["sa_Wqkv"], f32)
    sa_bqkv = np.asarray(inp["sa_bqkv"], f32)
    sa_Wo = np.asarray(inp["sa_Wo"], f32)
    sa_bo = np.asarray(inp["sa_bo"], f32)
    ln_b = np.asarray(inp["ln_b"], f32)
    ln2_b = np.asarray(inp["ln2_b"], f32)
    ln3_b = np.asarray(inp["ln3_b"], f32)
    ff_W1 = np.asarray(inp["ff_W1"], f32)
    ff_b1 = np.asarray(inp["ff_b1"], f32)
    ff_b2 = np.asarray(inp["ff_b2"], f32)
    fc_W = np.asarray(inp["fc_W"], f32)

    # beta_in[i]: the beta of the LN feeding block i (folded out of that LN)
    beta_in = np.stack([ln_b] + [ln3_b[i] for i in range(NB - 1)])

    wq = np.stack([_ksplit(sa_Wqkv[i, 0] * scale) for i in range(NB)])
    wk = np.stack([_ksplit(sa_Wqkv[i, 1]) for i in range(NB)])
    wv = np.stack([_ksplit(sa_Wqkv[i, 2]) for i in range(NB)])
    wo = np.stack([_ksplit(sa_Wo[i], p=DH) for i in range(NB)])
    w1 = np.stack([_ksplit(ff_W1[i]) for i in range(NB)])
    w2 = np.stack([_ksplit(np.asarray(inp["ff_W2"], f32)[i]) for i in range(NB)])

    bq = np.stack([(sa_bqkv[i, 0] + beta_in[i] @ sa_Wqkv[i, 0]) * scale
                   for i in range(NB)])
    bk = np.stack([sa_bqkv[i, 1] + beta_in[i] @ sa_Wqkv[i, 1]
                   for i in range(NB)])
    bv = np.stack([sa_bqkv[i, 2] + beta_in[i] @ sa_Wqkv[i, 2]
                   for i in range(NB)])
    bo = np.stack([sa_bo[i] + bv[i] @ sa_Wo[i] + beta_in[i]
                   for i in range(NB)])
    b1 = np.stack([ff_b1[i] + ln2_b[i] @ ff_W1[i] for i in range(NB)])
    b2 = np.stack([ff_b2[i] + ln2_b[i] for i in range(NB)])
    fcb = np.asarray(inp["fc_b"], f32) + ln3_b[NB - 1] @ fc_W

    task_table = np.asarray(inp["task_table"], f32)
    ca_Wqkv = np.asarray(inp["ca_Wqkv"], f32)
    ca_bqkv = np.asarray(inp["ca_bqkv"], f32)
    ca_Wo = np.asarray(inp["ca_Wo"], f32)
    ca_bo = np.asarray(inp["ca_bo"], f32)
    ln1_b = np.asarray(inp["ln1_b"], f32)
    enc = task_table[np.asarray(t)[:, 0]]  # [B, D]
    cab = np.zeros((NB, B, D), f32)
    for i in range(NB):
        v_ = enc @ ca_Wqkv[i, 2] + ca_bqkv[i, 2]
        cab[i] = v_ @ ca_Wo[i] + ca_bo[i] + ln1_b[i]

    ln1_g = np.asarray(inp["ln1_g"], f32)
    ln2_g = np.asarray(inp["ln2_g"], f32)
    ln3_g = np.asarray(inp["ln3_g"], f32)
    ln_g = np.asarray(inp["ln_g"], f32)
    lnp0_arr = np.stack([ln_g, -ln_g])

    pcol = np.arange(128)[:, None]
    ucol = np.arange(896)[None, :]
    masks = np.where(pcol > ucol - 384, f32(-30000.0), f32(0.0))

    def cols(v, p=128):
        """[X] -> [p, X//p] per-partition column layout (col-major k-split)."""
        return v.reshape(-1, p).T

    bmeta_all = []
    for core in range(NCORES):
        bmeta = np.zeros((NB, 128, _BMETA_COLS), f32)
        for i in range(NB):
            bmeta[i, 0:DH, _BQ:_BQ + H] = cols(bq[i], DH)
            bmeta[i, 0:DH, _BK:_BK + H] = cols(bk[i], DH)
            bmeta[i, :, _BO:_BO + MT] = cols(bo[i])
            bmeta[i, :, _B2:_B2 + MT] = cols(b2[i])
            for b in range(CPC):
                bmeta[i, :, _CAB + b * KT:_CAB + (b + 1) * KT] = \
                    cols(cab[i, core * CPC + b])
            bmeta[i, :, _L1G:_L1G + KT] = cols(ln1_g[i])
            bmeta[i, :, _L1N:_L1N + KT] = cols(-ln1_g[i])
            bmeta[i, :, _L2G:_L2G + KT] = cols(ln2_g[i])
            bmeta[i, :, _L2N:_L2N + KT] = cols(-ln2_g[i])
            bmeta[i, :, _L3G:_L3G + KT] = cols(ln3_g[i])
            bmeta[i, :, _L3N:_L3N + KT] = cols(-ln3_g[i])
            bmeta[i, :, _B1:_B1 + FFT] = cols(b1[i])
        bmeta_all.append(bmeta)

    posT = np.ascontiguousarray(_posenc(L, D).T)  # [D, L]
    pos_pk = np.ascontiguousarray(
        posT.reshape(KT, 128, L).transpose(1, 0, 2)).astype(bf)

    wr_ = np.asarray(inp["Wr"], f32)  # [1, 256]
    wrp_ = np.ascontiguousarray(wr_.reshape(2, 128).T)  # [128, 2]

    shared = dict(
        wa=np.asarray(inp["Wa"], f32).astype(bf),
        wrp=wrp_,
        ws=np.asarray(inp["Ws"], f32).astype(bf),
        bemb=np.concatenate([np.asarray(inp["ba"], f32),
                             np.asarray(inp["br"], f32),
                             np.asarray(inp["bs"], f32)]),
        lnp0=lnp0_arr,
        pos=pos_pk,
        wq=wq.astype(bf), wk=wk.astype(bf), wv=wv.astype(bf),
        wo=wo.astype(bf), w1=w1.astype(bf), w2=w2.astype(bf),
        masks=masks.astype(bf),
        fcw=fc_W.astype(bf),
        fcb=fcb,
    )
    in_maps = []
    for core in range(NCORES):
        m = dict(shared)
        m["ars"] = ars[core * CPC:(core + 1) * CPC]
        m["bmeta"] = bmeta_all[core]
        in_maps.append(m)
    return in_maps


def _get_nc(reps=1):
    key = f"nc{reps}"
    if key not in _CACHE:
        _CACHE[key] = _build(reps)
    return _CACHE[key]


def kernel(**inputs):
    nc = _get_nc()
    in_maps = _host_prep(inputs)
    res = run_bass_kernel_spmd(nc, in_maps, core_ids=list(range(NCORES)))
    out = np.zeros((B, L, A_DIM), np.float32)
    for core in range(NCORES):
        yc = res.results[core]["y"]  # [CPC, 64, L]
        for b in range(CPC):
            out[core * CPC + b] = yc[b].T
    return out


# revision 68
# speedup vs baseline: 1.0340x; 1.0340x over previous
"""Trainium2 Bass kernel for nn_DiscreteDecisionTransformer (v2).

Decision-transformer forward: embed(a,r,s) -> LN -> +posenc, then 4 blocks of
[causal self-attn, cross-attn, FFN] with post-LN, then action head.

Distribution: data-parallel over batch, 16 batches / 8 cores = 2 per core.
Params replicated; zero collectives. Feature-major layout ([dmodel on
partitions, tokens on free]) so GEMMs contract over partitions.

v2 changes vs the original working kernel (sim: 3.09ms -> 2.50ms):
 - Single-role residual stream: LayerNorm affine + residual adds are in-place,
   halving the X footprint.
 - LN betas folded into downstream linear biases wherever the next consumer is
   linear (LN0->block0 qkv + bo, LN2->b1/b2, LN3->next qkv + bo, last->fc_b).
   Only LN1 keeps an explicit beta (it feeds LN2, a nonlinearity).
 - V-projection bias folded into bo (bv @ Wo) -- the softmax denominator trick
   makes per-token V-bias contribution = bv exactly; r-embedding done as a
   rank-1 broadcast*scale on ACT instead of a matmul.
 - Batched DMAs: per-block weights land in 1 DMA each ([128, kt, cols] APs);
   FFN weights stream in 16 DMAs per (batch, chunk) instead of 120. This cut
   the serial HWDGE descriptor-generation time from 1.36ms to 0.23ms.
 - PSUM->SBUF bias-adds moved to the Activation engine (per-partition bias AP),
   LN affine and stats run bf16; 1/D folded into the stats matmul constant.
 - Emission-order tuning so in-order engine queues overlap: Q projected
   per-chunk inside attention, LN3 deferred into the next block's prologue,
   (batch, chunk) pairs interleaved through LN1/LN2/FFN, split A/B gpsimd
   broadcasts. Embed-phase scratch aliases the idle FFN hidden-tile pool.

GEMMs in bf16 with f32 PSUM accumulation (fp8 DoubleRow was measured at
rel-err 3.9e-2 vs the 2e-2 budget -- not viable). End-to-end error vs the
f32 reference is ~7e-3.
"""

import sys
from contextlib import ExitStack

sys.path.insert(0, "/opt/trn_rl_repo")

import numpy as np
import ml_dtypes

import concourse.bacc as bacc
import concourse.mybir as mybir
import concourse.tile as tile
from concourse.bass_utils import run_bass_kernel_spmd

bf = ml_dtypes.bfloat16

B, L, D, H, DH, NB, E = 16, 1024, 768, 8, 96, 4, 256
A_DIM, S_DIM = 64, 128
NCORES = 8
CPC = B // NCORES  # batches per core
KT = D // 128      # 6 k-tiles of dmodel
MT = D // 128      # 6 m-tiles of dmodel
CH = 512           # token chunk for attention / LN
NCH = L // CH      # 2 chunks per batch
FFT = 4 * D // 128 # 24 m-tiles of ffn hidden
F32, BF = mybir.dt.float32, mybir.dt.bfloat16
AL = mybir.AluOpType
AF = mybir.ActivationFunctionType

_CACHE = {}

# bmeta column layout (f32, per block): per-partition packed scalars
_BQ = 0          # [96, 8]   q bias (scaled)
_BK = 8          # [96, 8]   k bias
_BO = 16         # [128, 6]  o bias (+bv@Wo +beta_in)
_B2 = 22         # [128, 6]  ffn2 bias (+beta2)
_CAB = 28        # [128, 6*CPC] cross-attn bias + ln1 beta, per batch
_L1G = 40        # [128, 6]  ln1 gamma
_L1N = 46        # [128, 6]  -ln1 gamma
_L2G = 52
_L2N = 58
_L3G = 64
_L3N = 70
_B1 = 76         # [128, 24] ffn1 bias (+beta2@W1)
_BMETA_COLS = 100


def _rearr_pk(ap, p):
    return ap.rearrange("(k p) -> p k", p=p)


def _build(reps=1):
    nc = bacc.Bacc("TRN2", target_bir_lowering=False, debug=False)
    dram = nc.dram_tensor

    ars = dram("ars", [CPC, 193, L], BF, kind="ExternalInput")
    wa = dram("wa", [A_DIM, E], BF, kind="ExternalInput")
    wrp = dram("wrp", [128, 2], F32, kind="ExternalInput")
    ws = dram("ws", [S_DIM, E], BF, kind="ExternalInput")
    bemb = dram("bemb", [D], F32, kind="ExternalInput")
    lnp0 = dram("lnp0", [2, D], F32, kind="ExternalInput")  # g, -g (beta folded)
    pos = dram("pos", [128, KT, L], BF, kind="ExternalInput")
    wq = dram("wq", [NB, 128, KT, D], BF, kind="ExternalInput")
    wk = dram("wk", [NB, 128, KT, D], BF, kind="ExternalInput")
    wv = dram("wv", [NB, 128, KT, D], BF, kind="ExternalInput")
    wo = dram("wo", [NB, DH, H, D], BF, kind="ExternalInput")
    w1 = dram("w1", [NB, 128, KT, 4 * D], BF, kind="ExternalInput")
    w2 = dram("w2", [NB, 128, FFT, D], BF, kind="ExternalInput")
    bmeta = dram("bmeta", [NB, 128, _BMETA_COLS], F32, kind="ExternalInput")
    masks = dram("masks", [128, 896], BF, kind="ExternalInput")
    fcw = dram("fcw", [D, A_DIM], BF, kind="ExternalInput")
    fcb = dram("fcb", [A_DIM], F32, kind="ExternalInput")
    y = dram("y", [CPC, A_DIM, L], F32, kind="ExternalOutput")

    with nc.allow_low_precision(reason="bf16 kernel by design"), \
         tile.TileContext(nc) as tc, ExitStack() as ctx:
            ep = ctx.enter_context
            cst = ep(tc.tile_pool(name="cst", bufs=1))
            wblk = ep(tc.tile_pool(name="wblk", bufs=1))
            bmp = ep(tc.tile_pool(name="bmp", bufs=2))
            w1p = ep(tc.tile_pool(name="w1p", bufs=4))
            w2p = ep(tc.tile_pool(name="w2p", bufs=3))
            xp = ep(tc.tile_pool(name="xp", bufs=1))
            qkp = ep(tc.tile_pool(name="qk", bufs=1))
            vap = ep(tc.tile_pool(name="vap", bufs=1))
            ptp = ep(tc.tile_pool(name="ptp", bufs=8))
            otp = ep(tc.tile_pool(name="otp", bufs=1))
            scr = ep(tc.tile_pool(name="scr", bufs=3))
            hp = ep(tc.tile_pool(name="hp", bufs=1))
            smv = ep(tc.tile_pool(name="smv", bufs=3))
            abp = ep(tc.tile_pool(name="abp", bufs=2))
            bias = ep(tc.tile_pool(name="bias", bufs=1))
            pmm = ep(tc.tile_pool(name="pmm", bufs=6, space="PSUM"))

            # ---------- global constants ----------
            ones = cst.tile([128, 1], BF)
            nc.any.memset(ones[:], 1.0)
            ones_d = cst.tile([128, 1], BF)
            nc.any.memset(ones_d[:], 1.0 / D)
            epst = cst.tile([1, 1], F32)
            nc.any.memset(epst[:], 1e-5)
            bigm = cst.tile([128, 896], BF, tag="bigm")
            nc.sync.dma_start(bigm[:], masks[:])
            maskt = [bigm[:, 384 - rt * 128:896 - rt * 128] for rt in range(4)]
            fct = cst.tile([128, KT, A_DIM], BF, tag="fcw")
            nc.sync.dma_start(fct[:], fcw[:].rearrange("(k p) m -> p k m", p=128))
            fcbt = cst.tile([A_DIM, 1], F32, tag="fcb")
            nc.sync.dma_start(fcbt[:], fcb[:].rearrange("(m o) -> m o", o=1))

            # residual-stream tiles (single role, in-place updates)
            xt = [[xp.tile([128, L], BF, tag=f"x{b}_{k}", name=f"x{b}_{k}")
                   for k in range(KT)] for b in range(CPC)]

            def ln_chunk(c, X, gt, gnt, bt_=None, post_pos=False):
                """In-place LayerNorm (no beta unless bt_ given) over features
                for one 512-token chunk. X: 6 [128, L] bf16 tiles."""
                cs = slice(c * CH, (c + 1) * CH)
                if post_pos:
                    # embed-only: borrow the (idle) FFN hidden tiles
                    postc = [hp.tile([128, CH], BF, tag=f"h{k}",
                                     name=f"pos{k}") for k in range(KT)]
                    for k in range(KT):
                        nc.sync.dma_start(postc[k][:], pos[:, k, cs])
                # shares the PSUM bank with attention's `po` (phase-disjoint)
                st = pmm.tile([DH + 1, CH], F32, tag="po", bufs=2)
                # ones_d = 1/D, so the matmuls produce mu / E[x^2] directly
                for k in range(KT):
                    nc.tensor.matmul(st[0:1, :], ones_d[:], X[k][:, cs],
                                     start=(k == 0), stop=(k == KT - 1))
                for k in range(KT):
                    xsq = scr.tile([128, CH], BF, tag="xsq", bufs=3)
                    nc.scalar.activation(xsq[:], X[k][:, cs], AF.Square)
                    nc.tensor.matmul(st[32:33, :], ones_d[:], xsq[:],
                                     start=(k == 0), stop=(k == KT - 1))
                musm = smv.tile([1, 2 * CH], F32, tag="musm", bufs=2)
                nc.vector.tensor_copy(musm[:, 0:CH], st[0:1, :])
                nc.vector.tensor_copy(musm[:, CH:2 * CH], st[32:33, :])
                mu = musm[:, 0:CH]
                mu2 = smv.tile([1, CH], F32, tag="sm")
                nc.vector.tensor_mul(mu2[:], mu, mu)
                var = smv.tile([1, CH], F32, tag="sm")
                nc.vector.tensor_sub(var[:], musm[:, CH:2 * CH], mu2[:])
                sd = smv.tile([1, CH], F32, tag="sm")
                nc.scalar.activation(sd[:], var[:], AF.Sqrt, bias=epst[:])
                ab = abp.tile([1, 2 * CH], BF, tag="ab", bufs=1)
                nc.vector.reciprocal(ab[:, 0:CH], sd[:])
                abb = abp.tile([128, 2 * CH], BF, tag="abb")
                nc.gpsimd.partition_broadcast(abb[:, 0:CH], ab[:, 0:CH])
                nc.vector.tensor_mul(ab[:, CH:2 * CH], mu, ab[:, 0:CH])
                nc.gpsimd.partition_broadcast(abb[:, CH:2 * CH],
                                              ab[:, CH:2 * CH])
                for k in range(KT):
                    u = scr.tile([128, CH], BF, tag="scr")
                    nc.vector.scalar_tensor_tensor(
                        u[:], X[k][:, cs], gt[:, k:k + 1], abb[:, 0:CH],
                        op0=AL.mult, op1=AL.mult)
                    if post_pos:
                        w_ = scr.tile([128, CH], BF, tag="scr")
                        nc.vector.scalar_tensor_tensor(
                            w_[:], abb[:, CH:2 * CH], gnt[:, k:k + 1], u[:],
                            op0=AL.mult, op1=AL.add)
                        nc.vector.tensor_add(X[k][:, cs], w_[:],
                                             postc[k][:])
                    else:
                        nc.vector.scalar_tensor_tensor(
                            X[k][:, cs], abb[:, CH:2 * CH], gnt[:, k:k + 1],
                            u[:], op0=AL.mult, op1=AL.add)
                        if bt_ is not None:
                            nc.scalar.activation(X[k][:, cs], X[k][:, cs],
                                                 AF.Identity,
                                                 bias=bt_[:, k:k + 1])

            def emit_forward():
                # ---------- embed + LN + posenc ----------
                wat = cst.tile([A_DIM, E], BF, tag="wa")
                nc.sync.dma_start(wat[:], wa[:])
                wrt = cst.tile([128, 2], F32, tag="wr")
                nc.sync.dma_start(wrt[:], wrp[:])
                wst = cst.tile([S_DIM, E], BF, tag="ws")
                nc.sync.dma_start(wst[:], ws[:])
                bembt = cst.tile([128, KT], F32, tag="bemb")
                nc.sync.dma_start(bembt[:], _rearr_pk(bemb[:], 128))
                p0 = []
                for j in range(2):
                    t = cst.tile([128, KT], F32, tag=f"lnp0{j}")
                    nc.sync.dma_start(t[:], _rearr_pk(lnp0[j], 128))
                    p0.append(t)

                for b in range(CPC):
                    X = xt[b]
                    # embed inputs borrow idle FFN hidden tiles / ab tile
                    tr = abp.tile([1, 2 * CH], BF, tag="ab", bufs=1,
                                  name="tr")
                    nc.sync.dma_start(tr[:], ars[b, A_DIM:A_DIM + 1, :])
                    for c in range(NCH):
                        cc = slice(c * CH, (c + 1) * CH)
                        ta = hp.tile([128, CH], BF, tag="h6", name="ta")
                        nc.sync.dma_start(ta[0:A_DIM, :], ars[b, 0:A_DIM, cc])
                        ts = hp.tile([128, CH], BF, tag="h7", name="ts")
                        nc.sync.dma_start(ts[:], ars[b, A_DIM + 1:193, cc])
                        rb = hp.tile([128, CH], BF, tag="h8", name="rb")
                        nc.gpsimd.partition_broadcast(rb[:], tr[:, cc])
                        for m in range(MT):
                            ms = slice((m % 2) * 128, (m % 2) * 128 + 128)
                            if 2 <= m < 4:
                                # r-embedding is rank-1: X[m] = r*wr + bemb
                                nc.scalar.activation(
                                    X[m][:, cc], rb[:], AF.Identity,
                                    bias=bembt[:, m:m + 1],
                                    scale=wrt[:, m - 2:m - 1])
                                continue
                            p = pmm.tile([128, CH], F32, tag="mm")
                            if m < 2:
                                nc.tensor.matmul(p[:], wat[:, ms],
                                                 ta[0:A_DIM, :],
                                                 start=True, stop=True)
                            else:
                                nc.tensor.matmul(p[:], wst[:, ms], ts[:],
                                                 start=True, stop=True)
                            nc.scalar.activation(X[m][:, cc], p[:],
                                                 AF.Identity,
                                                 bias=bembt[:, m:m + 1])
                    for c in range(NCH):
                        ln_chunk(c, X, p0[0], p0[1], post_pos=True)

                # ---------- transformer blocks ----------
                # LN3 of batch b is emitted lazily, right before the next
                # consumer of X[b], so its DVE chain overlaps other batches'
                # PE work instead of stalling the in-order PE queue.
                ln3_pending = [None] * CPC
                for i in range(NB):
                    wqt = wblk.tile([128, KT, D], BF, tag="wq")
                    nc.sync.dma_start(wqt[:], wq[i])
                    wkt = wblk.tile([128, KT, D], BF, tag="wk")
                    nc.sync.dma_start(wkt[:], wk[i])
                    wvt = wblk.tile([128, KT, D], BF, tag="wv")
                    nc.sync.dma_start(wvt[:], wv[i])
                    wot = wblk.tile([DH, H, D], BF, tag="wo")
                    nc.sync.dma_start(wot[:], wo[i])
                    bm = bmp.tile([128, _BMETA_COLS], F32, tag="bm")
                    nc.sync.dma_start(bm[:], bmeta[i])

                    # flush deferred LN3s up front: b0's chain must finish
                    # before K-proj(b0), but b1's overlaps attention(b0)
                    for b in range(CPC):
                        if ln3_pending[b] is not None:
                            pg, pn = ln3_pending[b]
                            for c in range(NCH):
                                ln_chunk(c, xt[b], pg, pn)
                            ln3_pending[b] = None

                    for b in range(CPC):
                        X = xt[b]
                        # ---- K projection, full L (Q is done per chunk) ----
                        kt_ = []
                        for h in range(H):
                            tk = qkp.tile([DH, L], BF, tag=f"k{h}")
                            kt_.append(tk)
                            hs = slice(h * DH, (h + 1) * DH)
                            for c in range(NCH):
                                cc = slice(c * CH, (c + 1) * CH)
                                pk = pmm.tile([DH, CH], F32, tag="mm")
                                for k in range(KT):
                                    nc.tensor.matmul(pk[:], wkt[:, k, hs],
                                                     X[k][:, cc],
                                                     start=(k == 0),
                                                     stop=(k == KT - 1))
                                nc.scalar.activation(
                                    tk[:, cc], pk[:], AF.Identity,
                                    bias=bm[0:DH, _BK + h:_BK + h + 1])
                        # ---- V projection (token-major, bias folded away) --
                        vt = []
                        for tg in range(L // 128):
                            tok = slice(tg * 128, (tg + 1) * 128)
                            tv = vap.tile([128, 8 * 97], BF, tag=f"v{tg}")
                            for hg in range(2):
                                pv = pmm.tile([128, 4 * DH], F32, tag="mm")
                                for k in range(KT):
                                    nc.tensor.matmul(
                                        pv[:], X[k][:, tok],
                                        wvt[:, k, hg * 4 * DH:(hg + 1) * 4 * DH],
                                        start=(k == 0), stop=(k == KT - 1))
                                nc.scalar.activation(
                                    tv[:, hg * 4 * 97:(hg + 1) * 4 * 97]
                                    .rearrange("p (h d) -> p h d", d=97)
                                    [:, :, 0:DH],
                                    pv[:].rearrange("p (h d) -> p h d", d=DH),
                                    AF.Identity)
                            nc.vector.memset(tv[:, 96:8 * 97:97], 1.0)
                            vt.append(tv)
                        # ---- attention + O-proj per chunk ----
                        for c in range(NCH):
                            cs = slice(c * CH, (c + 1) * CH)
                            ktc = 4 * (c + 1)
                            ot = []
                            for h in range(H):
                                hs = slice(h * DH, (h + 1) * DH)
                                pq = pmm.tile([DH, CH], F32, tag="mm")
                                for k in range(KT):
                                    nc.tensor.matmul(pq[:], wqt[:, k, hs],
                                                     X[k][:, cs],
                                                     start=(k == 0),
                                                     stop=(k == KT - 1))
                                tq = qkp.tile([DH, CH], BF, tag="q", bufs=4)
                                nc.scalar.activation(
                                    tq[:], pq[:], AF.Identity,
                                    bias=bm[0:DH, _BQ + h:_BQ + h + 1])
                                po = pmm.tile([DH + 1, CH], F32, tag="po",
                                              bufs=2)
                                for g0 in range(0, ktc, 4):
                                    pts = []
                                    for kt2 in range(g0, g0 + 4):
                                        ks2 = slice(kt2 * 128, (kt2 + 1) * 128)
                                        ptile = ptp.tile([128, CH], BF,
                                                         tag="pt")
                                        rt = kt2 - 4 * c
                                        if rt < 0:
                                            psc = pmm.tile([128, CH], F32,
                                                           tag="mm")
                                            nc.tensor.matmul(
                                                psc[:], kt_[h][:, ks2], tq[:],
                                                start=True, stop=True)
                                            nc.scalar.activation(
                                                ptile[:], psc[:], AF.Exp)
                                            pts.append(ptile)
                                            continue
                                        psc = pmm.tile([128, CH], F32,
                                                       tag="mm")
                                        nc.tensor.matmul(
                                            psc[:], kt_[h][:, ks2], tq[:],
                                            start=True, stop=True)
                                        tmp = scr.tile([128, CH], F32,
                                                       tag="scrf", bufs=2)
                                        nc.vector.scalar_tensor_tensor(
                                            tmp[:], psc[:], 1.0, maskt[rt],
                                            op0=AL.mult, op1=AL.add)
                                        nc.scalar.activation(
                                            ptile[:], tmp[:], AF.Exp)
                                        pts.append(ptile)
                                    for j, ptile in enumerate(pts):
                                        kt2 = g0 + j
                                        nc.tensor.matmul(
                                            po[:],
                                            vt[kt2][:, h * 97:h * 97 + 97],
                                            ptile[:],
                                            start=(kt2 == 0),
                                            stop=(kt2 == ktc - 1))
                                dinv = abp.tile([1, CH], BF, tag="dv",
                                                name="dinv", bufs=1)
                                nc.vector.reciprocal(dinv[:], po[DH:DH + 1, :])
                                dib = abp.tile([DH, CH], BF, tag="dib", bufs=3)
                                nc.gpsimd.partition_broadcast(dib[:], dinv[:])
                                oht = otp.tile([DH, CH], BF, tag=f"o{h}",
                                               name=f"o{h}")
                                nc.vector.scalar_tensor_tensor(
                                    oht[:], po[0:DH, :], 1.0, dib[:],
                                    op0=AL.mult, op1=AL.mult)
                                ot.append(oht)
                            for m in range(MT):
                                ms = slice(m * 128, (m + 1) * 128)
                                pp = pmm.tile([128, CH], F32, tag="mm")
                                for h in range(H):
                                    nc.tensor.matmul(pp[:], wot[:, h, ms],
                                                     ot[h][:],
                                                     start=(h == 0),
                                                     stop=(h == H - 1))
                                nc.vector.scalar_tensor_tensor(
                                    X[m][:, cs], pp[:], bm[:, _BO + m:_BO + m + 1],
                                    X[m][:, cs], op0=AL.add, op1=AL.add)
                    # interleave (b, c) pairs: each FFN's matmuls overlap the
                    # next pair's LN chains on the vector engine. b0's LN3 is
                    # emitted mid-block (overlapping FFN(b1,c1) matmuls); b1's
                    # stays deferred to the next block's prologue.
                    for b, c in ((0, 0), (1, 0), (0, 1), (1, 1)):
                        X = xt[b]
                        if True:
                            cs = slice(c * CH, (c + 1) * CH)
                            # LN1 (beta = cross-attn bias + ln1_b), in place
                            ln_chunk(c, X, bm[:, _L1G:_L1G + KT],
                                     bm[:, _L1N:_L1N + KT],
                                     bt_=bm[:, _CAB + b * KT:_CAB + (b + 1) * KT])
                            # LN2 (beta folded into b1/b2), in place
                            ln_chunk(c, X, bm[:, _L2G:_L2G + KT],
                                     bm[:, _L2N:_L2N + KT])
                            # ---- FFN ----
                            ht = [hp.tile([128, CH], BF, tag=f"h{m}",
                                          name=f"h{m}") for m in range(FFT)]
                            for mg in range(FFT // 2):
                                colg = slice(mg * 256, (mg + 1) * 256)
                                w1g = w1p.tile([128, KT, 256], BF, tag="w1")
                                nc.sync.dma_start(w1g[:], w1[i, :, :, colg])
                                for mi in range(2):
                                    m = mg * 2 + mi
                                    p1 = pmm.tile([128, CH], F32, tag="mm")
                                    for k in range(KT):
                                        nc.tensor.matmul(
                                            p1[:],
                                            w1g[:, k, mi * 128:(mi + 1) * 128],
                                            X[k][:, cs],
                                            start=(k == 0), stop=(k == KT - 1))
                                    nc.scalar.activation(
                                        ht[m][:], p1[:], AF.Relu,
                                        bias=bm[:, _B1 + m:_B1 + m + 1])
                            for grp in range(2):
                                mcols = slice(grp * 384, (grp + 1) * 384)
                                p2s = [pmm.tile([128, CH], F32, tag="mm",
                                                name=f"p2_{mi}")
                                       for mi in range(3)]
                                for j in range(4):
                                    w2g = w2p.tile([128, FFT // 4, 384], BF,
                                                   tag="w2")
                                    nc.sync.dma_start(
                                        w2g[:],
                                        w2[i][:, j * 6:(j + 1) * 6, mcols])
                                    for k6 in range(FFT // 4):
                                        k = j * 6 + k6
                                        for mi in range(3):
                                            nc.tensor.matmul(
                                                p2s[mi][:],
                                                w2g[:, k6,
                                                    mi * 128:(mi + 1) * 128],
                                                ht[k][:],
                                                start=(k == 0),
                                                stop=(k == FFT - 1))
                                for mi in range(3):
                                    m = grp * 3 + mi
                                    nc.vector.scalar_tensor_tensor(
                                        X[m][:, cs], p2s[mi][:],
                                        bm[:, _B2 + m:_B2 + m + 1],
                                        X[m][:, cs], op0=AL.add, op1=AL.add)
                        # LN3 deferred (beta folded into next block / fc)
                        ln3_pending[b] = (bm[:, _L3G:_L3G + KT],
                                          bm[:, _L3N:_L3N + KT])

                # ---------- action head ----------
                # flush both LN3s first: head(b0) matmuls then overlap b1's
                # LN3 chain on the vector engine
                for b in range(CPC):
                    if ln3_pending[b] is not None:
                        pg, pn = ln3_pending[b]
                        for c in range(NCH):
                            ln_chunk(c, xt[b], pg, pn)
                        ln3_pending[b] = None
                for b in range(CPC):
                    X = xt[b]
                    for c in range(NCH):
                        cs = slice(c * CH, (c + 1) * CH)
                        pf = pmm.tile([A_DIM, CH], F32, tag="mm")
                        for k in range(KT):
                            nc.tensor.matmul(pf[:], fct[:, k, :], X[k][:, cs],
                                             start=(k == 0), stop=(k == KT - 1))
                        yt = scr.tile([128, CH], F32, tag="scrf", bufs=2)
                        nc.vector.tensor_scalar_add(yt[0:A_DIM, :], pf[:],
                                                    fcbt[:])
                        nc.sync.dma_start(y[b, :, cs], yt[0:A_DIM, :])

            for _rep in range(reps):
                emit_forward()

    nc.compile()
    return nc


def _posenc(length, d):
    pos_ = np.arange(length, dtype=np.float32)[:, None]
    i = np.arange(0, d, 2, dtype=np.float32)[None, :]
    ang = pos_ / np.power(np.float32(10000.0), i / np.float32(d))
    pe = np.zeros((length, d), np.float32)
    pe[:, 0::2] = np.sin(ang)
    pe[:, 1::2] = np.cos(ang)
    return pe


def _ksplit(w, p=128):
    """[K, F] -> [p, K//p, F] with k = t*p + row."""
    k, f = w.shape
    return np.ascontiguousarray(w.reshape(k // p, p, f).transpose(1, 0, 2))


def _host_prep(inp):
    f32 = np.float32
    a, r, s, t = (np.asarray(inp[k]) for k in ("a", "r", "s", "t"))
    ars = np.concatenate(
        [np.asarray(a, f32), np.asarray(r, f32), np.asarray(s, f32)],
        axis=-1).transpose(0, 2, 1)  # [B, 193, L]
    ars = np.ascontiguousarray(ars).astype(bf)

    scale = f32(1.0 / np.sqrt(DH))
    sa_Wqkv = np.asarray(inp["sa_Wqkv"], f32)
    sa_bqkv = np.asarray(inp["sa_bqkv"], f32)
    sa_Wo = np.asarray(inp["sa_Wo"], f32)
    sa_bo = np.asarray(inp["sa_bo"], f32)
    ln_b = np.asarray(inp["ln_b"], f32)
    ln2_b = np.asarray(inp["ln2_b"], f32)
    ln3_b = np.asarray(inp["ln3_b"], f32)
    ff_W1 = np.asarray(inp["ff_W1"], f32)
    ff_b1 = np.asarray(inp["ff_b1"], f32)
    ff_b2 = np.asarray(inp["ff_b2"], f32)
    fc_W = np.asarray(inp["fc_W"], f32)

    # beta_in[i]: the beta of the LN feeding block i (folded out of that LN)
    beta_in = np.stack([ln_b] + [ln3_b[i] for i in range(NB - 1)])

    wq = np.stack([_ksplit(sa_Wqkv[i, 0] * scale) for i in range(NB)])
    wk = np.stack([_ksplit(sa_Wqkv[i, 1]) for i in range(NB)])
    wv = np.stack([_ksplit(sa_Wqkv[i, 2]) for i in range(NB)])
    wo = np.stack([_ksplit(sa_Wo[i], p=DH) for i in range(NB)])
    w1 = np.stack([_ksplit(ff_W1[i]) for i in range(NB)])
    w2 = np.stack([_ksplit(np.asarray(inp["ff_W2"], f32)[i]) for i in range(NB)])

    bq = np.stack([(sa_bqkv[i, 0] + beta_in[i] @ sa_Wqkv[i, 0]) * scale
                   for i in range(NB)])
    bk = np.stack([sa_bqkv[i, 1] + beta_in[i] @ sa_Wqkv[i, 1]
                   for i in range(NB)])
    bv = np.stack([sa_bqkv[i, 2] + beta_in[i] @ sa_Wqkv[i, 2]
                   for i in range(NB)])
    bo = np.stack([sa_bo[i] + bv[i] @ sa_Wo[i] + beta_in[i]
                   for i in range(NB)])
    b1 = np.stack([ff_b1[i] + ln2_b[i] @ ff_W1[i] for i in range(NB)])
    b2 = np.stack([ff_b2[i] + ln2_b[i] for i in range(NB)])
    fcb = np.asarray(inp["fc_b"], f32) + ln3_b[NB - 1] @ fc_W

    task_table = np.asarray(inp["task_table"], f32)
    ca_Wqkv = np.asarray(inp["ca_Wqkv"], f32)
    ca_bqkv = np.asarray(inp["ca_bqkv"], f32)
    ca_Wo = np.asarray(inp["ca_Wo"], f32)
    ca_bo = np.asarray(inp["ca_bo"], f32)
    ln1_b = np.asarray(inp["ln1_b"], f32)
    enc = task_table[np.asarray(t)[:, 0]]  # [B, D]
    cab = np.zeros((NB, B, D), f32)
    for i in range(NB):
        v_ = enc @ ca_Wqkv[i, 2] + ca_bqkv[i, 2]
        cab[i] = v_ @ ca_Wo[i] + ca_bo[i] + ln1_b[i]

    ln1_g = np.asarray(inp["ln1_g"], f32)
    ln2_g = np.asarray(inp["ln2_g"], f32)
    ln3_g = np.asarray(inp["ln3_g"], f32)
    ln_g = np.asarray(inp["ln_g"], f32)
    lnp0_arr = np.stack([ln_g, -ln_g])

    pcol = np.arange(128)[:, None]
    ucol = np.arange(896)[None, :]
    masks = np.where(pcol > ucol - 384, f32(-30000.0), f32(0.0))

    def cols(v, p=128):
        """[X] -> [p, X//p] per-partition column layout (col-major k-split)."""
        return v.reshape(-1, p).T

    bmeta_all = []
    for core in range(NCORES):
        bmeta = np.zeros((NB, 128, _BMETA_COLS), f32)
        for i in range(NB):
            bmeta[i, 0:DH, _BQ:_BQ + H] = cols(bq[i], DH)
            bmeta[i, 0:DH, _BK:_BK + H] = cols(bk[i], DH)
            bmeta[i, :, _BO:_BO + MT] = cols(bo[i])
            bmeta[i, :, _B2:_B2 + MT] = cols(b2[i])
            for b in range(CPC):
                bmeta[i, :, _CAB + b * KT:_CAB + (b + 1) * KT] = \
                    cols(cab[i, core * CPC + b])
            bmeta[i, :, _L1G:_L1G + KT] = cols(ln1_g[i])
            bmeta[i, :, _L1N:_L1N + KT] = cols(-ln1_g[i])
            bmeta[i, :, _L2G:_L2G + KT] = cols(ln2_g[i])
            bmeta[i, :, _L2N:_L2N + KT] = cols(-ln2_g[i])
            bmeta[i, :, _L3G:_L3G + KT] = cols(ln3_g[i])
            bmeta[i, :, _L3N:_L3N + KT] = cols(-ln3_g[i])
            bmeta[i, :, _B1:_B1 + FFT] = cols(b1[i])
        bmeta_all.append(bmeta)

    posT = np.ascontiguousarray(_posenc(L, D).T)  # [D, L]
    pos_pk = np.ascontiguousarray(
        posT.reshape(KT, 128, L).transpose(1, 0, 2)).astype(bf)

    wr_ = np.asarray(inp["Wr"], f32)  # [1, 256]
    wrp_ = np.ascontiguousarray(wr_.reshape(2, 128).T)  # [128, 2]

    shared = dict(
        wa=np.asarray(inp["Wa"], f32).astype(bf),
        wrp=wrp_,
        ws=np.asarray(inp["Ws"], f32).astype(bf),
        bemb=np.concatenate([np.asarray(inp["ba"], f32),
                             np.asarray(inp["br"], f32),
                             np.asarray(inp["bs"], f32)]),
        lnp0=lnp0_arr,
        pos=pos_pk,
        wq=wq.astype(bf), wk=wk.astype(bf), wv=wv.astype(bf),
        wo=wo.astype(bf), w1=w1.astype(bf), w2=w2.astype(bf),
        masks=masks.astype(bf),
        fcw=fc_W.astype(bf),
        fcb=fcb,
    )
    in_maps = []
    for core in range(NCORES):
        m = dict(shared)
        m["ars"] = ars[core * CPC:(core + 1) * CPC]
        m["bmeta"] = bmeta_all[core]
        in_maps.append(m)
    return in_maps


def _get_nc(reps=1):
    key = f"nc{reps}"
    if key not in _CACHE:
        _CACHE[key] = _build(reps)
    return _CACHE[key]


def kernel(**inputs):
    nc = _get_nc()
    in_maps = _host_prep(inputs)
    res = run_bass_kernel_spmd(nc, in_maps, core_ids=list(range(NCORES)))
    out = np.zeros((B, L, A_DIM), np.float32)
    for core in range(NCORES):
        yc = res.results[core]["y"]  # [CPC, 64, L]
        for b in range(CPC):
            out[core * CPC + b] = yc[b].T
    return out
